# revision 24
# baseline (speedup 1.0000x reference)
"""Position-attention kernel for Trainium2 (8 NeuronCores, Bass/Tile).

Module: q,k = 1x1 convs to C/8 channels, v = 1x1 conv, attn = softmax(q^T k),
y = v @ attn^T, out = gamma*y + x.  Shapes: B=4, C=512, H=W=64 (N=4096, Cq=64).

Sharding: data-parallel over batch x query-halves -> 8 cores. Core i handles
batch i//2, query positions [h*2048, (h+1)*2048) with h = i%2. Each core
computes full K/V projections for its batch and its half of Q, then
S^T = k^T q in [key, query] layout, exp to fp8e5, and y = v @ attn^T via
vT-stationary DoubleRow matmuls.

v2 changes vs the original baseline:
  - All projections run as fp8e4 DoubleRow matmuls (x and weights in fp8,
    256-channel contraction per instruction) -> half the projection matmuls.
  - The softmax denominator is accumulated on the PE with an all-ones
    [128,2,128] fp8 DoubleRow matmul alongside the U matmuls: every PSUM
    partition receives sum_keys(e), so no DVE accumulation chain and no
    gpsimd partition-reduce.
  - Residual is a bf16 input with gamma*v_b folded host-side (no 4MB f32
    xr tensor); output is written bf16 and upcast on the host.
  - Epilogue runs per c-tile straight out of PSUM (reciprocal -> mul ->
    add residual on DVE, pipelined with the out-DMA), shrinking the tail.

Per-core key permutation puts the core's own query half first so one SPMD
program works for both halves.
"""

import numpy as np
import ml_dtypes

import concourse.bass as bass
import concourse.mybir as mybir
import concourse.tile as tile
from concourse import bacc
from concourse.bass_utils import run_bass_kernel_spmd

BF16 = ml_dtypes.bfloat16
FP8E4 = ml_dtypes.float8_e4m3

B, C, H, W = 4, 512, 64, 64
N = H * W            # 4096 keys per batch
NQ = N // 2          # 2048 queries per core
CQ = C // 8          # 64 q/k channels
P = 128
CT = C // P          # 4 channel tiles
MT = N // P          # 32 key tiles
NCH = 512            # matmul moving-dim chunk
QCH = NQ // NCH      # 4 query chunks per core
KCH = N // NCH       # 8 key chunks
NCORES = 8

F32 = mybir.dt.float32
F16 = mybir.dt.float16
BF = mybir.dt.bfloat16
F8 = mybir.dt.float8e4
F8E = mybir.dt.float8e5
AF = mybir.ActivationFunctionType
DR = mybir.MatmulPerfMode.DoubleRow
LN16 = 2.772588722239781  # exp shift (ln 16): E in fp8e5m2, max logit ~10.9 -> e^8.1 ~ 3300 < 57344

_CACHE = {}


def _build_program():
    nc = bacc.Bacc()

    # DRAM inputs. x8/kw8/qw8/vw8 are pre-packed host-side to the DoubleRow
    # channel layout [p, pair, j, *] with channel = pair*256 + j*128 + p.
    # x8 is packed host-side as [p, chunk, pair, j, 512] so every 512-column
    # chunk DMA reads 2KB contiguous per partition (fast descriptors).
    x8 = nc.declare_dram_parameter("x8", [P, 4 * N], F8, isOutput=False)
    xres = nc.declare_dram_parameter("xres", [C, NQ], BF, isOutput=False)
    kw8 = nc.declare_dram_parameter("kw8", [P, 4 * CQ], F8, isOutput=False)
    qw8 = nc.declare_dram_parameter("qw8", [P, 4 * CQ], F8, isOutput=False)
    vw8 = nc.declare_dram_parameter("vw8", [P, 4 * C], F8, isOutput=False)
    qb = nc.declare_dram_parameter("qb", [CQ, 1], F32, isOutput=False)
    kb = nc.declare_dram_parameter("kb", [CQ, 1], F32, isOutput=False)
    out = nc.declare_dram_parameter("out", [C, NQ], BF, isOutput=True)

    with tile.TileContext(nc) as tc:
        with tc.tile_pool(name="consts", bufs=1) as consts:
            x8_sb = consts.tile([P, KCH, 2, 2, NCH], F8)
            xres_sb = consts.tile([P, CT * NQ], BF)
            kw8_sb = consts.tile([P, 2, 2, CQ], F8)
            qw8_sb = consts.tile([P, 2, 2, CQ], F8)
            vw8_sb = consts.tile([P, 2, 2, C], F8)
            qb_sb = consts.tile([CQ, 1], F32)
            kb_sb = consts.tile([CQ, 1], F32)
            # zero-padded to 128 partitions so st matmuls run in the same
            # (128,128) PE tiling mode as the DoubleRow matmuls.
            q_sb = consts.tile([P, NQ], BF)
            k_sb = consts.tile([P, N], BF)
            vt_sb = consts.tile([P, MT * C], F8)    # vT: 32 m-tiles of [128, 512]
            ones8 = consts.tile([P, 2, P], F8)      # warm-up operand
            ones16 = consts.tile([P, P], F16)       # partition-reduce lhsT for the denominator

            x8_r = x8[:, :].rearrange("p (c a j n) -> p c a j n", c=KCH, a=2, j=2)

            # Input DMAs split across the two HWDGE queues (sync, scalar),
            # ordered by when the projections need each x-column range and
            # balanced in bytes across the two rings.
            # The 16 HW DMA queues round-robin across every in-flight
            # dma_start, so the first-needed tensor only wins if nothing
            # else is queued: x8 chunk 0 leads BOTH rings (half each), and
            # everything not needed in the first ~15us (x8 chunks 2-7,
            # xres) is issued from the gpsimd ring *after* its memsets so
            # it can't compete early.
            nc.sync.dma_start(out=x8_sb[:64, 0], in_=x8_r[:64, 0])
            nc.scalar.dma_start(out=x8_sb[64:, 0], in_=x8_r[64:, 0])
            nc.sync.dma_start(out=kw8_sb, in_=kw8[:, :].rearrange("p (a j o) -> p a j o", a=2, j=2))
            nc.scalar.dma_start(out=qw8_sb, in_=qw8[:, :].rearrange("p (a j o) -> p a j o", a=2, j=2))
            nc.sync.dma_start(out=x8_sb[:64, 1], in_=x8_r[:64, 1])
            nc.scalar.dma_start(out=x8_sb[64:, 1], in_=x8_r[64:, 1])
            nc.scalar.dma_start(out=qb_sb, in_=qb[:, :])
            nc.sync.dma_start(out=kb_sb, in_=kb[:, :])
            nc.sync.dma_start(out=vw8_sb, in_=vw8[:, :].rearrange("p (a j o) -> p a j o", a=2, j=2))

            ln16_sb = consts.tile([P, 1], F32)
            nc.vector.memset(ones8, 1.0)
            nc.vector.memset(ones16, 1.0)
            nc.vector.memset(ln16_sb, -LN16)
            # pad zeroing on the otherwise-idle gpsimd so the DVE is free for
            # the k/q/v drains as soon as the first projections land
            nc.gpsimd.memset(k_sb[CQ:, :], 0.0)
            nc.gpsimd.memset(q_sb[CQ:, :], 0.0)
            for ch in range(2, KCH):
                nc.gpsimd.dma_start(out=x8_sb[:, ch], in_=x8_r[:, ch])
            xres_sb_r = xres_sb.rearrange("p (t m) -> p t m", t=CT)
            xres_r = xres[:, :].rearrange("(t p) m -> p t m", p=P)
            nc.gpsimd.dma_start(out=xres_sb_r[:, :2], in_=xres_r[:, :2])
            nc.gpsimd.dma_start(out=xres_sb_r[:, 2:], in_=xres_r[:, 2:])

            # HAM warm-up: >=3.4us of sustained dummy matmuls while the x8
            # DMA is in flight, so the PE clock gate is at 8/8 by the time
            # the real projections start, with no >3.4us idle gap before
            # the first real matmul.
            with tc.tile_pool(name="warm_ps", bufs=1, space="PSUM") as warm_ps:
                wt = warm_ps.tile([P, 2 * P], F32, name="warm")
                ones_flat = ones8.rearrange("p a b -> p (a b)")
                for _ in range(30):
                    nc.tensor.matmul(wt, lhsT=ones8[:, 0, :], rhs=ones_flat,
                                     start=True, stop=True)

            # bias APs broadcast along the free dim for the DVE k/q drains
            kb_b = bass.AP(tensor=kb_sb.tensor, offset=kb_sb.offset,
                           ap=[kb_sb.ap[0], [0, NCH]])
            qb_b = bass.AP(tensor=qb_sb.tensor, offset=qb_sb.offset,
                           ap=[qb_sb.ap[0], [0, NCH]])

            # ---- projections (all fp8 DoubleRow, 256-channel contraction) ----
            vt3 = vt_sb.rearrange("p (m c) -> p m c", m=MT)
            with (
                tc.tile_pool(name="kq_ps", bufs=2, space="PSUM") as kq_ps,
                tc.tile_pool(name="v_ps", bufs=2, space="PSUM") as v_ps,
            ):
                def k_proj(ch):
                    kp = kq_ps.tile([CQ, NCH], F32, tag="kq", name="kp")
                    for pair in range(2):
                        nc.tensor.matmul(
                            kp, lhsT=kw8_sb[:, pair, :, :],
                            rhs=x8_sb[:, ch, pair],
                            start=(pair == 0), stop=(pair == 1), perf_mode=DR)
                    nc.vector.tensor_add(k_sb[:CQ, ch * NCH:(ch + 1) * NCH], kp, kb_b)

                def q_proj(ch):
                    qp = kq_ps.tile([CQ, NCH], F32, tag="kq", name="qp")
                    for pair in range(2):
                        nc.tensor.matmul(
                            qp, lhsT=qw8_sb[:, pair, :, :],
                            rhs=x8_sb[:, ch, pair],
                            start=(pair == 0), stop=(pair == 1), perf_mode=DR)
                    nc.vector.tensor_add(q_sb[:CQ, ch * NCH:(ch + 1) * NCH], qp, qb_b)

                def v_proj_pair(i):
                    # two key-tiles' worth of vT accumulated into a 2-bank
                    # PSUM tile, drained with one wide copy. Drains alternate
                    # ACT/DVE so neither engine becomes the projection-phase
                    # straggler (a late drain blocks the main loop's first st
                    # matmul through a PSUM-bank WAR hazard).
                    vp = v_ps.tile([P, 2, C], F32, tag="v", name="vp")
                    for sub in range(2):
                        mt = 2 * i + sub
                        for pair in range(2):
                            nc.tensor.matmul(
                                vp[:, sub, :],
                                lhsT=x8_sb[:, mt // 4, pair, :,
                                           (mt % 4) * P:(mt % 4 + 1) * P],
                                rhs=vw8_sb[:, pair, :, :],
                                start=(pair == 0), stop=(pair == 1), perf_mode=DR)
                    if i % 2 == 0:
                        nc.vector.tensor_copy(vt3[:, 2 * i:2 * i + 2, :], vp)
                    else:
                        nc.scalar.activation(vt3[:, 2 * i:2 * i + 2, :], vp, AF.Copy)

                # emitted in x-column-arrival order
                k_proj(0); q_proj(0)
                v_proj_pair(0); v_proj_pair(1)
                for ch in range(1, 4):
                    k_proj(ch); q_proj(ch)
                    v_proj_pair(2 * ch); v_proj_pair(2 * ch + 1)
                for ch in range(4, KCH):
                    k_proj(ch)
                    v_proj_pair(2 * ch); v_proj_pair(2 * ch + 1)

            # ---- attention main loop ----
            xres3 = xres_sb.rearrange("p (c m) -> p c m", c=CT)
            out_r = out[:, :].rearrange("(c p) n -> p c n", p=P)
            with (
                tc.tile_pool(name="st_ps", bufs=3, space="PSUM") as st_ps,
                tc.tile_pool(name="u_ps", bufs=4, space="PSUM") as u_ps,
                tc.tile_pool(name="d_ps", bufs=1, space="PSUM") as d_ps,
                tc.tile_pool(name="e_pool", bufs=8) as e_pool,
                tc.tile_pool(name="cs_pool", bufs=2) as cs_pool,
                tc.tile_pool(name="fin", bufs=2) as fin,
                tc.tile_pool(name="outp", bufs=4) as outp,
            ):
                def make_epilogue(us, cs, ch):
                    def epilogue():
                        # fold the two cs halves, partition-reduce on the PE
                        # with an all-ones stationary, then normalize + add
                        # residual per c-tile, pipelined with the out-DMA.
                        csf = fin.tile([P, NCH], F16, tag="csf", name="csf")
                        nc.vector.tensor_add(csf, cs[:, :NCH], cs[:, NCH:])
                        d = d_ps.tile([P, NCH], F32, tag="d", name="d")
                        nc.tensor.matmul(d, lhsT=ones16, rhs=csf,
                                         start=True, stop=True)
                        rec = fin.tile([P, NCH], F32, tag="rec", name="rec")
                        nc.vector.reciprocal_approx_fast(out=rec, in_=d)
                        # muls run one c-tile ahead of the adds: each mul
                        # frees a u PSUM bank the next chunk's first U
                        # matmul group is about to reuse.
                        tcs = [fin.tile([P, NCH], BF, tag="tc", name="tc")
                               for _ in range(CT)]

                        def mul(c):
                            nc.vector.tensor_mul(tcs[c], us[c], rec)

                        def add(c):
                            ob = outp.tile([P, NCH], BF, tag="ob", name="ob")
                            nc.vector.tensor_add(
                                ob, tcs[c], xres3[:, c, ch * NCH:(ch + 1) * NCH])
                            eng = nc.sync if c % 2 == 0 else nc.scalar
                            eng.dma_start(out=out_r[:, c, ch * NCH:(ch + 1) * NCH],
                                          in_=ob)

                        mul(0)
                        for c in range(1, CT):
                            mul(c)
                            add(c - 1)
                        add(CT - 1)
                    return epilogue

                pending_epilogue = None
                for ch in range(QCH):
                    us = [u_ps.tile([P, NCH], F32, tag="u", name=f"u{c}")
                          for c in range(CT)]
                    # fp16 running sum of e over key tiles (two halves, one
                    # per e2 row) at the DVE 16-bit 2x rate; partition-reduce
                    # happens once per chunk on the PE.
                    cs = cs_pool.tile([P, 2 * NCH], F16, tag="cs", name="cs")
                    qs = q_sb[:, ch * NCH:(ch + 1) * NCH]

                    sts = {}

                    def emit_st(mt, _qs=qs):
                        st = st_ps.tile([P, NCH], F32, tag="st", name="st")
                        nc.tensor.matmul(st, lhsT=k_sb[:, mt * P:(mt + 1) * P],
                                         rhs=_qs, start=True, stop=True)
                        sts[mt] = st

                    emit_st(0)
                    emit_st(1)
                    emit_st(2)
                    if pending_epilogue is not None:
                        # the previous chunk's epilogue goes out after this
                        # chunk's first st matmuls so the PE is not stalled
                        # on the DVE fold feeding the denominator matmul.
                        pending_epilogue()
                    for t in range(MT // 2):
                        e2 = e_pool.tile([P, 2, NCH], F8E, tag="e", name="e2")
                        for j in range(2):
                            mt = 2 * t + j
                            nc.scalar.activation(e2[:, j, :], sts.pop(mt), AF.Exp,
                                                 bias=ln16_sb)
                            if mt + 3 < MT:
                                emit_st(mt + 3)
                        e2f = e2.rearrange("p j n -> p (j n)")
                        if t == 0:
                            nc.vector.tensor_copy(cs, e2f)
                        else:
                            nc.vector.tensor_add(cs, cs, e2f)
                        first, last = t == 0, t == MT // 2 - 1
                        for c in range(CT):
                            nc.tensor.matmul(
                                us[c],
                                lhsT=vt3[:, 2 * t:2 * t + 2, c * P:(c + 1) * P],
                                rhs=e2,
                                start=first, stop=last, perf_mode=DR)
                    pending_epilogue = make_epilogue(us, cs, ch)
                pending_epilogue()
    nc.finalize()
    return nc


def _get_program():
    if "nc" not in _CACHE:
        _CACHE["nc"] = _build_program()
    return _CACHE["nc"]


def _pack_w(wT, out_cols):
    # [C, out] channel-major -> [p, pair, j, out] flat, channel = pair*256 + j*128 + p
    w4 = wT.reshape(2, 2, P, out_cols)
    return np.ascontiguousarray(w4.transpose(2, 0, 1, 3)).reshape(P, 4 * out_cols)


def make_in_maps(x, q_w, q_b, k_w, k_b, v_w, v_b, gamma):
    x = np.asarray(x, dtype=np.float32)
    gamma_f = float(np.asarray(gamma).reshape(-1)[0])
    qw8 = _pack_w(np.asarray(q_w, np.float32).T, CQ).astype(FP8E4)
    kw8 = _pack_w(np.asarray(k_w, np.float32).T, CQ).astype(FP8E4)
    vw8 = _pack_w(gamma_f * np.asarray(v_w, np.float32).T, C).astype(FP8E4)
    qb_c = np.asarray(q_b, np.float32).reshape(CQ, 1)
    kb_c = np.asarray(k_b, np.float32).reshape(CQ, 1)
    gvb = (gamma_f * np.asarray(v_b, np.float32)).reshape(C, 1)

    xf = x.reshape(B, C, N)
    in_maps = []
    for core in range(NCORES):
        b, h = core // 2, core % 2
        mine = xf[b, :, h * NQ:(h + 1) * NQ]
        other = xf[b, :, (1 - h) * NQ:(2 - h) * NQ]
        x_perm = np.concatenate([mine, other], axis=1)
        # [p, chunk, pair, j, 512]: channel = pair*256 + j*128 + p
        x8 = np.ascontiguousarray(
            x_perm.reshape(2, 2, P, KCH, NCH).transpose(2, 3, 0, 1, 4)
        ).reshape(P, 4 * N).astype(FP8E4)
        in_maps.append({
            "x8": x8,
            "xres": (mine + gvb).astype(BF16),
            "qw8": qw8, "kw8": kw8, "vw8": vw8,
            "qb": qb_c, "kb": kb_c,
        })
    return in_maps


def run(in_maps, **kwargs):
    nc = _get_program()
    return run_bass_kernel_spmd(nc, in_maps, list(range(NCORES)), **kwargs)


def gather(results):
    out = np.empty((B, C, N), dtype=np.float32)
    for core in range(NCORES):
        b, h = core // 2, core % 2
        out[b, :, h * NQ:(h + 1) * NQ] = results[core]["out"].astype(np.float32)
    return out.reshape(B, C, H, W)


def kernel(x, q_w, q_b, k_w, k_b, v_w, v_b, gamma, **_):
    in_maps = make_in_maps(x, q_w, q_b, k_w, k_b, v_w, v_b, gamma)
    res = run(in_maps)
    return gather(res.results)


# revision 27
# speedup vs baseline: 1.0078x; 1.0078x over previous
"""Position-attention kernel for Trainium2 (8 NeuronCores, Bass/Tile).

Module: q,k = 1x1 convs to C/8 channels, v = 1x1 conv, attn = softmax(q^T k),
y = v @ attn^T, out = gamma*y + x.  Shapes: B=4, C=512, H=W=64 (N=4096, Cq=64).

Sharding: data-parallel over batch x query-halves -> 8 cores. Core i handles
batch i//2, query positions [h*2048, (h+1)*2048) with h = i%2. Each core
computes full K/V projections for its batch and its half of Q, then
S^T = k^T q in [key, query] layout, exp to fp8e5, and y = v @ attn^T via
vT-stationary DoubleRow matmuls.

v2 changes vs the original baseline:
  - All projections run as fp8e4 DoubleRow matmuls (x and weights in fp8,
    256-channel contraction per instruction) -> half the projection matmuls.
  - The softmax denominator is accumulated on the PE with an all-ones
    [128,2,128] fp8 DoubleRow matmul alongside the U matmuls: every PSUM
    partition receives sum_keys(e), so no DVE accumulation chain and no
    gpsimd partition-reduce.
  - Residual is a bf16 input with gamma*v_b folded host-side (no 4MB f32
    xr tensor); output is written bf16 and upcast on the host.
  - Epilogue runs per c-tile straight out of PSUM (reciprocal -> mul ->
    add residual on DVE, pipelined with the out-DMA), shrinking the tail.

Per-core key permutation puts the core's own query half first so one SPMD
program works for both halves.
"""

import numpy as np
import ml_dtypes

import concourse.bass as bass
import concourse.mybir as mybir
import concourse.tile as tile
from concourse import bacc
from concourse.bass_utils import run_bass_kernel_spmd

BF16 = ml_dtypes.bfloat16
FP8E4 = ml_dtypes.float8_e4m3

B, C, H, W = 4, 512, 64, 64
N = H * W            # 4096 keys per batch
NQ = N // 2          # 2048 queries per core
CQ = C // 8          # 64 q/k channels
P = 128
CT = C // P          # 4 channel tiles
MT = N // P          # 32 key tiles
NCH = 512            # matmul moving-dim chunk
QCH = NQ // NCH      # 4 query chunks per core
KCH = N // NCH       # 8 key chunks
NCORES = 8

F32 = mybir.dt.float32
F16 = mybir.dt.float16
BF = mybir.dt.bfloat16
F8 = mybir.dt.float8e4
F8E = mybir.dt.float8e5
AF = mybir.ActivationFunctionType
DR = mybir.MatmulPerfMode.DoubleRow
LN16 = 2.772588722239781  # exp shift (ln 16): E in fp8e5m2, max logit ~10.9 -> e^8.1 ~ 3300 < 57344

_CACHE = {}


def _build_program():
    nc = bacc.Bacc()

    # DRAM inputs. x8/kw8/qw8/vw8 are pre-packed host-side to the DoubleRow
    # channel layout [p, pair, j, *] with channel = pair*256 + j*128 + p.
    # x8 is packed host-side as [p, chunk, pair, j, 512] so every 512-column
    # chunk DMA reads 2KB contiguous per partition (fast descriptors).
    x8 = nc.declare_dram_parameter("x8", [P, 4 * N], F8, isOutput=False)
    xres = nc.declare_dram_parameter("xres", [C, NQ], BF, isOutput=False)
    kw8 = nc.declare_dram_parameter("kw8", [P, 4 * CQ], F8, isOutput=False)
    qw8 = nc.declare_dram_parameter("qw8", [P, 4 * CQ], F8, isOutput=False)
    vw8 = nc.declare_dram_parameter("vw8", [P, 4 * C], F8, isOutput=False)
    qb = nc.declare_dram_parameter("qb", [CQ, 1], F32, isOutput=False)
    kb = nc.declare_dram_parameter("kb", [CQ, 1], F32, isOutput=False)
    out = nc.declare_dram_parameter("out", [C, NQ], BF, isOutput=True)

    with tile.TileContext(nc) as tc:
        with tc.tile_pool(name="consts", bufs=1) as consts:
            x8_sb = consts.tile([P, KCH, 2, 2, NCH], F8)
            xres_sb = consts.tile([P, CT * NQ], BF)
            kw8_sb = consts.tile([P, 2, 2, CQ], F8)
            qw8_sb = consts.tile([P, 2, 2, CQ], F8)
            vw8_sb = consts.tile([P, 2, 2, C], F8)
            qb_sb = consts.tile([CQ, 1], F32)
            kb_sb = consts.tile([CQ, 1], F32)
            # zero-padded to 128 partitions so st matmuls run in the same
            # (128,128) PE tiling mode as the DoubleRow matmuls.
            q_sb = consts.tile([P, NQ], BF)
            k_sb = consts.tile([P, N], BF)
            vt_sb = consts.tile([P, MT * C], F8)    # vT: 32 m-tiles of [128, 512]
            ones8 = consts.tile([P, 2, P], F8)      # all-ones DoubleRow lhsT for the denominator

            x8_r = x8[:, :].rearrange("p (c a j n) -> p c a j n", c=KCH, a=2, j=2)

            # Input DMAs split across the two HWDGE queues (sync, scalar),
            # ordered by when the projections need each x-column range and
            # balanced in bytes across the two rings.
            # The 16 HW DMA queues round-robin across every in-flight
            # dma_start, so the first-needed tensor only wins if nothing
            # else is queued: x8 chunk 0 leads BOTH rings (half each), and
            # everything not needed in the first ~15us (x8 chunks 2-7,
            # xres) is issued from the gpsimd ring *after* its memsets so
            # it can't compete early.
            nc.sync.dma_start(out=x8_sb[:64, 0], in_=x8_r[:64, 0])
            nc.scalar.dma_start(out=x8_sb[64:, 0], in_=x8_r[64:, 0])
            nc.sync.dma_start(out=kw8_sb, in_=kw8[:, :].rearrange("p (a j o) -> p a j o", a=2, j=2))
            nc.scalar.dma_start(out=qw8_sb, in_=qw8[:, :].rearrange("p (a j o) -> p a j o", a=2, j=2))
            nc.sync.dma_start(out=x8_sb[:64, 1], in_=x8_r[:64, 1])
            nc.scalar.dma_start(out=x8_sb[64:, 1], in_=x8_r[64:, 1])
            nc.scalar.dma_start(out=qb_sb, in_=qb[:, :])
            nc.sync.dma_start(out=kb_sb, in_=kb[:, :])
            nc.sync.dma_start(out=vw8_sb, in_=vw8[:, :].rearrange("p (a j o) -> p a j o", a=2, j=2))

            ln16_sb = consts.tile([P, 1], F32)
            nc.vector.memset(ones8, 1.0)
            nc.vector.memset(ln16_sb, -LN16)
            # pad zeroing on the otherwise-idle gpsimd so the DVE is free for
            # the k/q/v drains as soon as the first projections land
            nc.gpsimd.memset(k_sb[CQ:, :], 0.0)
            nc.gpsimd.memset(q_sb[CQ:, :], 0.0)
            for ch in range(2, KCH):
                nc.gpsimd.dma_start(out=x8_sb[:, ch], in_=x8_r[:, ch])
            xres_sb_r = xres_sb.rearrange("p (t m) -> p t m", t=CT)
            xres_r = xres[:, :].rearrange("(t p) m -> p t m", p=P)
            nc.gpsimd.dma_start(out=xres_sb_r[:, :2], in_=xres_r[:, :2])
            nc.gpsimd.dma_start(out=xres_sb_r[:, 2:], in_=xres_r[:, 2:])

            # HAM warm-up: >=3.4us of sustained dummy matmuls while the x8
            # DMA is in flight, so the PE clock gate is at 8/8 by the time
            # the real projections start, with no >3.4us idle gap before
            # the first real matmul.
            with tc.tile_pool(name="warm_ps", bufs=1, space="PSUM") as warm_ps:
                wt = warm_ps.tile([P, 2 * P], F32, name="warm")
                ones_flat = ones8.rearrange("p a b -> p (a b)")
                for _ in range(30):
                    nc.tensor.matmul(wt, lhsT=ones8[:, 0, :], rhs=ones_flat,
                                     start=True, stop=True)

            # bias APs broadcast along the free dim for the DVE k/q drains
            kb_b = bass.AP(tensor=kb_sb.tensor, offset=kb_sb.offset,
                           ap=[kb_sb.ap[0], [0, NCH]])
            qb_b = bass.AP(tensor=qb_sb.tensor, offset=qb_sb.offset,
                           ap=[qb_sb.ap[0], [0, NCH]])

            # ---- projections (all fp8 DoubleRow, 256-channel contraction) ----
            vt3 = vt_sb.rearrange("p (m c) -> p m c", m=MT)
            with (
                tc.tile_pool(name="kq_ps", bufs=2, space="PSUM") as kq_ps,
                tc.tile_pool(name="v_ps", bufs=2, space="PSUM") as v_ps,
            ):
                def k_proj(ch):
                    kp = kq_ps.tile([CQ, NCH], F32, tag="kq", name="kp")
                    for pair in range(2):
                        nc.tensor.matmul(
                            kp, lhsT=kw8_sb[:, pair, :, :],
                            rhs=x8_sb[:, ch, pair],
                            start=(pair == 0), stop=(pair == 1), perf_mode=DR)
                    nc.vector.tensor_add(k_sb[:CQ, ch * NCH:(ch + 1) * NCH], kp, kb_b)

                def q_proj(ch):
                    qp = kq_ps.tile([CQ, NCH], F32, tag="kq", name="qp")
                    for pair in range(2):
                        nc.tensor.matmul(
                            qp, lhsT=qw8_sb[:, pair, :, :],
                            rhs=x8_sb[:, ch, pair],
                            start=(pair == 0), stop=(pair == 1), perf_mode=DR)
                    nc.vector.tensor_add(q_sb[:CQ, ch * NCH:(ch + 1) * NCH], qp, qb_b)

                def v_proj_pair(i):
                    # two key-tiles' worth of vT accumulated into a 2-bank
                    # PSUM tile, drained with one wide copy. Drains alternate
                    # ACT/DVE so neither engine becomes the projection-phase
                    # straggler (a late drain blocks the main loop's first st
                    # matmul through a PSUM-bank WAR hazard).
                    vp = v_ps.tile([P, 2, C], F32, tag="v", name="vp")
                    for sub in range(2):
                        mt = 2 * i + sub
                        for pair in range(2):
                            nc.tensor.matmul(
                                vp[:, sub, :],
                                lhsT=x8_sb[:, mt // 4, pair, :,
                                           (mt % 4) * P:(mt % 4 + 1) * P],
                                rhs=vw8_sb[:, pair, :, :],
                                start=(pair == 0), stop=(pair == 1), perf_mode=DR)
                    if i % 2 == 0:
                        nc.vector.tensor_copy(vt3[:, 2 * i:2 * i + 2, :], vp)
                    else:
                        nc.scalar.activation(vt3[:, 2 * i:2 * i + 2, :], vp, AF.Copy)

                # emitted in x-column-arrival order
                k_proj(0); q_proj(0)
                v_proj_pair(0); v_proj_pair(1)
                for ch in range(1, 4):
                    k_proj(ch); q_proj(ch)
                    v_proj_pair(2 * ch); v_proj_pair(2 * ch + 1)
                for ch in range(4, KCH):
                    k_proj(ch)
                    v_proj_pair(2 * ch); v_proj_pair(2 * ch + 1)

            # ---- attention main loop ----
            xres3 = xres_sb.rearrange("p (c m) -> p c m", c=CT)
            out_r = out[:, :].rearrange("(c p) n -> p c n", p=P)
            with (
                tc.tile_pool(name="st_ps", bufs=3, space="PSUM") as st_ps,
                tc.tile_pool(name="u_ps", bufs=4, space="PSUM") as u_ps,
                tc.tile_pool(name="d_ps", bufs=1, space="PSUM") as d_ps,
                tc.tile_pool(name="e_pool", bufs=8) as e_pool,
                tc.tile_pool(name="fin", bufs=2) as fin,
                tc.tile_pool(name="outp", bufs=4) as outp,
            ):
                def make_epilogue(us, d, ch):
                    def epilogue():
                        # normalize + add residual per c-tile, pipelined
                        # with the out-DMA.
                        rec = fin.tile([P, NCH], F32, tag="rec", name="rec")
                        nc.vector.reciprocal_approx_fast(out=rec, in_=d)
                        # muls run one c-tile ahead of the adds: each mul
                        # frees a u PSUM bank the next chunk's first U
                        # matmul group is about to reuse.
                        tcs = [fin.tile([P, NCH], BF, tag="tc", name="tc")
                               for _ in range(CT)]

                        def mul(c):
                            nc.vector.tensor_mul(tcs[c], us[c], rec)

                        def add(c):
                            ob = outp.tile([P, NCH], BF, tag="ob", name="ob")
                            nc.vector.tensor_add(
                                ob, tcs[c], xres3[:, c, ch * NCH:(ch + 1) * NCH])
                            eng = nc.sync if c % 2 == 0 else nc.scalar
                            eng.dma_start(out=out_r[:, c, ch * NCH:(ch + 1) * NCH],
                                          in_=ob)

                        mul(0)
                        for c in range(1, CT):
                            mul(c)
                            add(c - 1)
                        add(CT - 1)
                    return epilogue

                pending_epilogue = None
                for ch in range(QCH):
                    us = [u_ps.tile([P, NCH], F32, tag="u", name=f"u{c}")
                          for c in range(CT)]
                    d = d_ps.tile([P, NCH], F32, tag="d", name="d")
                    qs = q_sb[:, ch * NCH:(ch + 1) * NCH]

                    sts = {}

                    def emit_st(mt, _qs=qs):
                        st = st_ps.tile([P, NCH], F32, tag="st", name="st")
                        nc.tensor.matmul(st, lhsT=k_sb[:, mt * P:(mt + 1) * P],
                                         rhs=_qs, start=True, stop=True)
                        sts[mt] = st

                    emit_st(0)
                    emit_st(1)
                    emit_st(2)
                    if pending_epilogue is not None:
                        # the previous chunk's epilogue goes out after this
                        # chunk's first st matmuls so its DVE work overlaps
                        # this chunk's pipeline ramp.
                        pending_epilogue()
                    for t in range(MT // 2):
                        e2 = e_pool.tile([P, 2, NCH], F8E, tag="e", name="e2")
                        for j in range(2):
                            mt = 2 * t + j
                            nc.scalar.activation(e2[:, j, :], sts.pop(mt), AF.Exp,
                                                 bias=ln16_sb)
                            if mt + 3 < MT:
                                emit_st(mt + 3)
                        first, last = t == 0, t == MT // 2 - 1
                        # denominator first so the reciprocal can start while
                        # the last U matmuls still run.
                        nc.tensor.matmul(d, lhsT=ones8, rhs=e2,
                                         start=first, stop=last, perf_mode=DR)
                        for c in range(CT):
                            nc.tensor.matmul(
                                us[c],
                                lhsT=vt3[:, 2 * t:2 * t + 2, c * P:(c + 1) * P],
                                rhs=e2,
                                start=first, stop=last, perf_mode=DR)
                    pending_epilogue = make_epilogue(us, d, ch)
                pending_epilogue()
    nc.finalize()
    return nc


def _get_program():
    if "nc" not in _CACHE:
        _CACHE["nc"] = _build_program()
    return _CACHE["nc"]


def _pack_w(wT, out_cols):
    # [C, out] channel-major -> [p, pair, j, out] flat, channel = pair*256 + j*128 + p
    w4 = wT.reshape(2, 2, P, out_cols)
    return np.ascontiguousarray(w4.transpose(2, 0, 1, 3)).reshape(P, 4 * out_cols)


def make_in_maps(x, q_w, q_b, k_w, k_b, v_w, v_b, gamma):
    x = np.asarray(x, dtype=np.float32)
    gamma_f = float(np.asarray(gamma).reshape(-1)[0])
    qw8 = _pack_w(np.asarray(q_w, np.float32).T, CQ).astype(FP8E4)
    kw8 = _pack_w(np.asarray(k_w, np.float32).T, CQ).astype(FP8E4)
    vw8 = _pack_w(gamma_f * np.asarray(v_w, np.float32).T, C).astype(FP8E4)
    qb_c = np.asarray(q_b, np.float32).reshape(CQ, 1)
    kb_c = np.asarray(k_b, np.float32).reshape(CQ, 1)
    gvb = (gamma_f * np.asarray(v_b, np.float32)).reshape(C, 1)

    xf = x.reshape(B, C, N)
    in_maps = []
    for core in range(NCORES):
        b, h = core // 2, core % 2
        mine = xf[b, :, h * NQ:(h + 1) * NQ]
        other = xf[b, :, (1 - h) * NQ:(2 - h) * NQ]
        x_perm = np.concatenate([mine, other], axis=1)
        # [p, chunk, pair, j, 512]: channel = pair*256 + j*128 + p
        x8 = np.ascontiguousarray(
            x_perm.reshape(2, 2, P, KCH, NCH).transpose(2, 3, 0, 1, 4)
        ).reshape(P, 4 * N).astype(FP8E4)
        in_maps.append({
            "x8": x8,
            "xres": (mine + gvb).astype(BF16),
            "qw8": qw8, "kw8": kw8, "vw8": vw8,
            "qb": qb_c, "kb": kb_c,
        })
    return in_maps


def run(in_maps, **kwargs):
    nc = _get_program()
    return run_bass_kernel_spmd(nc, in_maps, list(range(NCORES)), **kwargs)


def gather(results):
    out = np.empty((B, C, N), dtype=np.float32)
    for core in range(NCORES):
        b, h = core // 2, core % 2
        out[b, :, h * NQ:(h + 1) * NQ] = results[core]["out"].astype(np.float32)
    return out.reshape(B, C, H, W)


def kernel(x, q_w, q_b, k_w, k_b, v_w, v_b, gamma, **_):
    in_maps = make_in_maps(x, q_w, q_b, k_w, k_b, v_w, v_b, gamma)
    res = run(in_maps)
    return gather(res.results)
